# revision 55
# baseline (speedup 1.0000x reference)
"""Trainium2 Bass kernel for the Chebyshev atomic descriptor (gnn_message_passing).

Contract: kernel(**inputs) takes FULL unsharded inputs (positions [20000,3] f32,
species_idx [20000] i32, neighbor_idx [480000] i32) and returns the full
[20000, 52] f32 feature array.

Sharding: data-parallel over atoms. Each of the 8 NeuronCores owns 2560 atoms
(N padded 20000->20480); the position/typespin table is replicated per core and
each core's neighbor rows are fetched ON DEVICE with dma_gather (256B-row
vector gather, 30720 rows per supertile). Per-atom features stay sharded and
are concatenated on host.

Compute layout: atoms on partitions (G=10 atoms/partition/supertile). The j<k
angular triplets are mapped to a regular rectangle (circular distance d=1..12 x
i=0..23, d=12 half-weighted) so pair expansion is pure access patterns.
Chebyshev sums use weighted recurrences S_t = w*T_t (masked neighbors are exact
zeros); the typespin-weighted variants are sign flips (s in {-1,+1}) of the
unweighted chains. Work is split DVE (A-chains + segment reduces) / GPSIMD
(sign products, halving-tree pre-reduces, radial chains, gathers) / ACT
(sqrt/sin/affine), with DVE reduces of pool-produced data deferred a couple of
orders to avoid head-of-line stalls.
"""

import math
from contextlib import ExitStack

import numpy as np

import bass_rust
import concourse.bass as bass
import concourse.bacc as bacc
import concourse.tile as tile
from concourse import mybir
from concourse.bass_utils import run_bass_kernel_spmd

F32 = mybir.dt.float32
I32 = mybir.dt.int32
I16 = mybir.dt.int16
Alu = mybir.AluOpType
Act = mybir.ActivationFunctionType
AX = mybir.AxisListType

# ---- problem constants (hardcoded per harness contract) ----
N = 20000
K = 24
NCORES = 8
NPAD = 20480            # padded N, divisible by NCORES*128*G
NPC = NPAD // NCORES    # atoms per core = 2560
PT = 128                # partitions
G = 10                  # atoms per partition per supertile
SUP = NPC // (PT * G)   # supertiles per core = 2
KG = K * G              # neighbor slots per partition per supertile = 240
RAD_ORDER = 16
ANG_ORDER = 8
RAD_CUT = 8.0
ANG_CUT = 6.5
MIN_CUT = 0.55
DG = 12                 # circular-distance groups d=1..12
PAIR = DG * K           # 288 pair columns per atom (d=12 double-counted, half-weighted)
PAIRG = PAIR * G        # 2880
FEAT = 52
ROWE = 64               # padded table row: 64 f32 = 256B (dma_gather granularity)
GQ = 8                  # dma_gather calls per supertile
NRAD = RAD_ORDER + 1    # 17
NANG = ANG_ORDER + 1    # 9

HALF_PI = math.pi / 2.0
# x = 2*(d - MIN_CUT)/(RAD_CUT - MIN_CUT) - 1 = d*AX_ + BX_
AX_ = 2.0 / (RAD_CUT - MIN_CUT)
BX_ = -2.0 * MIN_CUT / (RAD_CUT - MIN_CUT) - 1.0


def view(ap, off, dims):
    """Custom free-dim view of a tile AP: keep the partition entry, replace the
    free dims with explicit [step, count] pairs (supports step-0 broadcasts and
    overlapping windows), shift the in-partition element offset by `off`."""
    base = list(ap.ap[0])
    return bass_rust.AP(ap.tensor, ap.offset + off, [base] + [list(d) for d in dims])


def build_supertile(nc, io, kp, app, s, pos4, idx16, pself, feat, half_pi, dbg=None):
    base = s * PT * G  # first atom (core-local) of this supertile

    # ---- loads ----
    ps = io.tile([PT, 4 * G], F32, tag="ps")
    nc.sync.dma_start(
        out=ps[:],
        in_=pself[base : base + PT * G, :].rearrange("(p g) c -> p (g c)", p=PT),
    )
    # gather neighbor [x,y,z,s] rows via dma_gather (256B table rows), then
    # compact the leading 16B of each gathered row into pn [PT, KG*4]
    pn = io.tile([PT, KG * 4], F32, tag="pn")
    CQ = KG // GQ  # gathered (g,k) chunks per dma_gather call
    NIDX = CQ * PT
    for q in range(GQ):
        idx_t = io.tile([PT, NIDX // 16], I16, tag="idx_t")
        row0 = (s * GQ + q) * PT
        nc.sync.dma_start(out=idx_t[:], in_=idx16[row0 : row0 + PT, :])
        pnw = io.tile([PT, CQ * ROWE], F32, tag="pnw")
        nc.gpsimd.dma_gather(
            out_ap=view(pnw[:], 0, [[ROWE, CQ], [1, ROWE]]),
            in_ap=pos4,
            idxs_ap=idx_t[:],
            num_idxs=NIDX,
            num_idxs_reg=NIDX,
            elem_size=ROWE,
            single_packet=False,
        )
        nc.scalar.copy(
            out=view(pn[:], q * CQ * 4, [[1, CQ * 4]]),
            in_=view(pnw[:], 0, [[ROWE, CQ], [1, 4]]),
        )

    # ---- K-space prep (free dims (G, K) = 240 elems) ----
    # rvec = pos_nbr - pos_self
    r_c = []
    for c in range(3):
        r = kp.tile([PT, KG], F32, tag=f"r{c}")
        nc.vector.tensor_tensor(
            out=r[:].rearrange("p (g k) -> p g k", g=G),
            in0=view(pn[:], c, [[4 * K, G], [4, K]]),
            in1=view(ps[:], c, [[4, G], [0, K]]),
            op=Alu.subtract,
        )
        r_c.append(r)
    # d2 = rx^2 + ry^2 + rz^2  (squares on ACT, adds on DVE)
    sq = []
    for c in range(3):
        q = kp.tile([PT, KG], F32, tag=f"sq{c}")
        nc.scalar.activation(q[:], r_c[c][:], Act.Square)
        sq.append(q)
    d2 = kp.tile([PT, KG], F32, tag="d2")
    nc.vector.tensor_tensor(out=d2[:], in0=sq[0][:], in1=sq[1][:], op=Alu.add)
    nc.vector.tensor_tensor(out=d2[:], in0=d2[:], in1=sq[2][:], op=Alu.add)
    # clamp to avoid rsqrt(0); masked-out anyway (d <= MIN_CUT)
    nc.vector.tensor_scalar_max(d2[:], d2[:], 1e-18)
    dd = kp.tile([PT, KG], F32, tag="dd")
    nc.scalar.sqrt(dd[:], d2[:])
    rinv = kp.tile([PT, KG], F32, tag="rinv")
    nc.vector.reciprocal(rinv[:], dd[:])

    # unit vectors into extended (wrap-around) buffers [G, 36]
    ue = []
    for c in range(3):
        e = kp.tile([PT, 36 * G], F32, tag=f"ue{c}")
        nc.vector.tensor_tensor(
            out=view(e[:], 0, [[36, G], [1, K]]),
            in0=r_c[c][:].rearrange("p (g k) -> p g k", g=G),
            in1=rinv[:].rearrange("p (g k) -> p g k", g=G),
            op=Alu.mult,
        )
        ue.append(e)

    # masks: m2 = (d > MIN_CUT); m1h = 0.5*(d <= RAD_CUT); a1h = 0.5*(d <= ANG_CUT)
    m2 = kp.tile([PT, KG], F32, tag="m2")
    nc.vector.tensor_scalar(
        out=m2[:], in0=dd[:], scalar1=MIN_CUT, scalar2=None, op0=Alu.is_gt
    )
    m1h = kp.tile([PT, KG], F32, tag="m1h")
    nc.vector.tensor_scalar(
        out=m1h[:], in0=dd[:], scalar1=RAD_CUT, scalar2=0.5, op0=Alu.is_le, op1=Alu.mult
    )
    a1h = kp.tile([PT, KG], F32, tag="a1h")
    nc.vector.tensor_scalar(
        out=a1h[:], in0=dd[:], scalar1=ANG_CUT, scalar2=0.5, op0=Alu.is_le, op1=Alu.mult
    )
    # cos cutoffs via sin(pi/2 - pi*min(d,rc)/rc) = cos(pi*d/rc) for in-mask d.
    # Clamping d at rc keeps the sin argument in [-pi/2, pi/2] (ACT table
    # domain); clamped out-of-mask values give fc=0 and are masked anyway.
    dcr = kp.tile([PT, KG], F32, tag="dcr")
    nc.vector.tensor_scalar_min(dcr[:], dd[:], RAD_CUT)
    grad = kp.tile([PT, KG], F32, tag="grad")
    nc.scalar.activation(
        grad[:], dcr[:], Act.Sin, bias=half_pi[:], scale=-math.pi / RAD_CUT
    )
    dca = kp.tile([PT, KG], F32, tag="dca")
    nc.vector.tensor_scalar_min(dca[:], dd[:], ANG_CUT)
    gang = kp.tile([PT, KG], F32, tag="gang")
    nc.scalar.activation(
        gang[:], dca[:], Act.Sin, bias=half_pi[:], scale=-math.pi / ANG_CUT
    )

    # radial weights wr = fc*m = mh*(grad+1) (DVE)
    Sr0a = kp.tile([PT, KG], F32, tag="Sr0a")
    wr = Sr0a[:]
    mh = kp.tile([PT, KG], F32, tag="mh")
    nc.vector.tensor_tensor(out=mh[:], in0=m1h[:], in1=m2[:], op=Alu.mult)
    nc.vector.tensor_tensor(out=wr, in0=mh[:], in1=grad[:], op=Alu.mult)
    nc.vector.tensor_tensor(out=wr, in0=wr, in1=mh[:], op=Alu.add)
    sn = view(pn[:], 3, [[4, KG]])  # neighbor typespin

    # angular per-neighbor weights fcm = fca*m_ang; neighbor spin (ext bufs)
    fcme = kp.tile([PT, 36 * G], F32, tag="fcme")
    se = kp.tile([PT, 36 * G], F32, tag="se")
    fcm_b = view(fcme[:], 0, [[36, G], [1, K]])
    mA = kp.tile([PT, KG], F32, tag="mA")
    nc.vector.tensor_tensor(out=mA[:], in0=a1h[:], in1=m2[:], op=Alu.mult)
    mA3 = mA[:].rearrange("p (g k) -> p g k", g=G)
    gang3 = gang[:].rearrange("p (g k) -> p g k", g=G)
    nc.vector.tensor_tensor(out=fcm_b, in0=mA3, in1=gang3, op=Alu.mult)
    nc.vector.tensor_tensor(out=fcm_b, in0=fcm_b, in1=mA3, op=Alu.add)
    nc.scalar.copy(
        view(se[:], 0, [[36, G], [1, K]]), view(pn[:], 3, [[4 * K, G], [4, K]])
    )

    # wrap-around copies: ext[:, 24:36] = ext[:, 0:12]
    for e in (*ue, fcme, se):
        nc.vector.tensor_copy(
            view(e[:], K, [[36, G], [1, 12]]), view(e[:], 0, [[36, G], [1, 12]])
        )

    # x map and 2x (on ACT)
    xx = kp.tile([PT, KG], F32, tag="xx")
    nc.scalar.activation(xx[:], dd[:], Act.Copy, bias=BX_, scale=AX_)
    x2 = kp.tile([PT, KG], F32, tag="x2")
    nc.scalar.activation(x2[:], xx[:], Act.Copy, scale=2.0)

    # feature accumulator
    featt = app.tile([PT, G * FEAT], F32, tag="featt")

    def rad_reduce(src_ap, col):
        nc.vector.tensor_reduce(
            out=view(featt[:], col, [[FEAT, G]]),
            in_=view(src_ap, 0, [[K, G], [1, K]]),
            axis=AX.X,
            op=Alu.add,
        )

    # Deferred DVE reduces: pool-produced reduce inputs are reduced on DVE a
    # couple of angular orders later so DVE never head-of-line blocks on pool.
    pending = []

    def defer(tag, fn):
        pending.append((tag, fn))

    def drain_deferred(now):
        rest = []
        for tag, fn in pending:
            if tag <= now:
                fn()
            else:
                rest.append((tag, fn))
        pending[:] = rest

    # ---- radial chains: S_t = wr*T_t(x) on pool; B values are sn * S_t.
    # Emitted interleaved with the angular loop (via emit_radial_order) so
    # pool radial work fills gaps while DVE runs the angular A chain.
    rtmpb = [kp.tile([PT, KG], F32, name=f"rtmpb{i}", tag=f"rtmpb{i}") for i in range(8)]
    rbuf = [Sr0a] + [
        kp.tile([PT, KG], F32, name=f"Sr{i}a", tag=f"Sr{i}a") for i in range(1, 8)
    ]
    rtmpa = [kp.tile([PT, KG], F32, name=f"rtmpa{i}", tag=f"rtmpa{i}") for i in range(2)]

    def rad_b(src, t, slot):
        rt = rtmpb[t % 8]
        nc.gpsimd.tensor_tensor(out=rt[:], in0=src[:], in1=sn, op=Alu.mult)
        defer(slot + 2, lambda rt=rt, t=t: rad_reduce(rt[:], NRAD + t))

    def emit_radial_order(t, slot):
        if t == 0:
            defer(slot + 2, lambda: rad_reduce(Sr0a[:], 0))
            rad_b(Sr0a, 0, slot)
        elif t == 1:
            nc.gpsimd.tensor_tensor(
                out=rbuf[1][:], in0=xx[:], in1=rbuf[0][:], op=Alu.mult
            )
            defer(slot + 2, lambda: rad_reduce(rbuf[1][:], 1))
            rad_b(rbuf[1], 1, slot)
        else:
            cur, prev, dst = rbuf[(t - 1) % 8], rbuf[(t - 2) % 8], rbuf[t % 8]
            ra = rtmpa[t % 2]
            nc.gpsimd.tensor_tensor(out=ra[:], in0=x2[:], in1=cur[:], op=Alu.mult)
            nc.gpsimd.tensor_tensor(
                out=dst[:], in0=ra[:], in1=prev[:], op=Alu.subtract
            )
            defer(slot + 2, lambda dst=dst, t=t: rad_reduce(dst[:], t))
            rad_b(dst, t, slot)

    # ---- angular: cos(theta) over pair rectangle (g, d=1..12, i=0..23) ----
    ct = app.tile([PT, PAIRG], F32, tag="ct")
    tp = app.tile([PT, PAIRG], F32, tag="atmp")  # aliases atmp (disjoint lifetime)
    ct3 = view(ct[:], 0, [[PAIR, G], [K, DG], [1, K]])
    tp3 = view(tp[:], 0, [[PAIR, G], [K, DG], [1, K]])
    for c in range(3):
        jj = view(ue[c][:], 0, [[36, G], [0, DG], [1, K]])
        kk = view(ue[c][:], 1, [[36, G], [1, DG], [1, K]])
        if c == 0:
            nc.vector.tensor_tensor(out=ct3, in0=jj, in1=kk, op=Alu.mult)
        else:
            nc.vector.tensor_tensor(out=tp3, in0=jj, in1=kk, op=Alu.mult)
            nc.vector.tensor_tensor(out=ct[:], in0=ct[:], in1=tp[:], op=Alu.add)
    c2t = app.tile([PT, PAIRG], F32, tag="c2t")
    nc.scalar.activation(c2t[:], ct[:], Act.Copy, scale=2.0)

    # pair weights w = fcm_j*fcm_k (d=12 halved) and pair sign ss = s_j*s_k
    Sa0 = app.tile([PT, PAIRG], F32, tag="Sa0")
    ss = app.tile([PT, PAIRG], F32, tag="ss")
    for eng, dst, e in ((nc.vector, Sa0, fcme), (nc.gpsimd, ss, se)):
        eng.tensor_tensor(
            out=view(dst[:], 0, [[PAIR, G], [K, DG], [1, K]]),
            in0=view(e[:], 0, [[36, G], [0, DG], [1, K]]),
            in1=view(e[:], 1, [[36, G], [1, DG], [1, K]]),
            op=Alu.mult,
        )
    dv = view(Sa0[:], (DG - 1) * K, [[PAIR, G], [1, K]])
    nc.vector.tensor_scalar(out=dv, in0=dv, scalar1=0.5, scalar2=None, op0=Alu.mult)

    def ang_reduce_a(src_ap, t):
        # chain A (unweighted): full 288-wide segment reduce on DVE
        nc.vector.tensor_reduce(
            out=view(featt[:], 2 * NRAD + t, [[FEAT, G]]),
            in_=view(src_ap, 0, [[PAIR, G], [1, PAIR]]),
            axis=AX.X,
            op=Alu.add,
        )

    def ang_reduce_b(src, scratch, t):
        # chain B: even t -> gpsimd halving tree 288->9 + tiny DVE tail;
        # odd t -> plain DVE segment reduce (balances the two engines).
        # All DVE parts are deferred 2 orders (inputs come from pool).
        if t % 2 == 1:
            defer(
                t + 3,
                lambda src=src, t=t: nc.vector.tensor_reduce(
                    out=view(featt[:], 2 * NRAD + NANG + t, [[FEAT, G]]),
                    in_=view(src[:], 0, [[PAIR, G], [1, PAIR]]),
                    axis=AX.X,
                    op=Alu.add,
                ),
            )
            return
        nc.gpsimd.tensor_tensor(
            out=view(scratch[:], 0, [[PAIR, G], [1, 144]]),
            in0=view(src[:], 0, [[PAIR, G], [1, 144]]),
            in1=view(src[:], 144, [[PAIR, G], [1, 144]]),
            op=Alu.add,
        )
        n = 144
        while n > 9:
            nc.gpsimd.tensor_tensor(
                out=view(scratch[:], 0, [[PAIR, G], [1, n // 2]]),
                in0=view(scratch[:], 0, [[PAIR, G], [1, n // 2]]),
                in1=view(scratch[:], n // 2, [[PAIR, G], [1, n // 2]]),
                op=Alu.add,
            )
            n //= 2
        defer(
            t + 3,
            lambda scratch=scratch, t=t: nc.vector.tensor_reduce(
                out=view(featt[:], 2 * NRAD + NANG + t, [[FEAT, G]]),
                in_=view(scratch[:], 0, [[PAIR, G], [1, 9]]),
                axis=AX.X,
                op=Alu.add,
            ),
        )

    # chain A (weights w) on DVE; B values are ss * S_t (sign flip only)
    btree = [
        app.tile([PT, PAIRG], F32, name="btree0", tag="btree0"),
        # btree1 first written at t=2, after ct's last read (the S1 mult)
        app.tile([PT, PAIRG], F32, name="btree1", tag="ct"),
    ]
    btmp = [app.tile([PT, PAIRG], F32, name=f"btmp{i}", tag=f"btmp{i}") for i in range(3)]

    def chain_b(src, t):
        bt = btmp[t % 3]
        nc.gpsimd.tensor_tensor(out=bt[:], in0=ss[:], in1=src[:], op=Alu.mult)
        ang_reduce_b(bt, btree[(t // 2) % 2], t)

    rad_emitted = 0

    def drain_radial(n, slot):
        nonlocal rad_emitted
        for _ in range(n):
            if rad_emitted < NRAD:
                emit_radial_order(rad_emitted, slot)
                rad_emitted += 1

    drain_radial(3, 0)
    ang_reduce_a(Sa0[:], 0)
    chain_b(Sa0, 0)
    abuf = [Sa0] + [
        app.tile([PT, PAIRG], F32, name=f"Sa{i}", tag=f"Sa{i}") for i in (1, 2, 3)
    ]
    nc.vector.tensor_tensor(out=abuf[1][:], in0=ct[:], in1=abuf[0][:], op=Alu.mult)
    ang_reduce_a(abuf[1][:], 1)
    chain_b(abuf[1], 1)
    drain_radial(2, 1)
    atmp = app.tile([PT, PAIRG], F32, tag="atmp")
    for t in range(2, NANG):
        drain_deferred(t)
        cur, prev, dst = abuf[(t - 1) % 4], abuf[(t - 2) % 4], abuf[t % 4]
        nc.vector.tensor_tensor(out=atmp[:], in0=c2t[:], in1=cur[:], op=Alu.mult)
        nc.vector.tensor_tensor(
            out=dst[:], in0=atmp[:], in1=prev[:], op=Alu.subtract
        )
        ang_reduce_a(dst[:], t)
        chain_b(dst, t)
        drain_radial(2, t)
    drain_radial(NRAD, NANG)
    drain_deferred(10**9)

    # ---- store ----
    nc.sync.dma_start(
        out=feat[base : base + PT * G, :].rearrange("(p g) f -> p (g f)", p=PT),
        in_=featt[:],
    )

    if dbg is not None:
        for name, t in [
            ("pn", pn), ("dd", dd), ("rinv", rinv), ("grad", grad),
            ("gang", gang), ("xx", xx), ("ue0", ue[0]),
            ("fcme", fcme), ("ct", ct),
        ]:
            d = dbg[name]
            w = d.shape[1] // SUP
            nc.sync.dma_start(out=d[:, s * w : (s + 1) * w], in_=t[:])


DBG_SHAPES = {
    "pn": KG * 4, "dd": KG, "rinv": KG, "grad": KG, "gang": KG,
    "xx": KG, "ue0": 36 * G, "fcme": 36 * G,
    "ct": PAIRG,
}


def build_program(debug=False):
    nc = bacc.Bacc(
        "TRN2",
        target_bir_lowering=False,
        debug=False,
    )
    pos4 = nc.dram_tensor("pos4", [NPAD, ROWE], F32, kind="ExternalInput").ap()
    idx16 = nc.dram_tensor(
        "idx16", [SUP * GQ * PT, KG // GQ * PT // 16], I16, kind="ExternalInput"
    ).ap()
    pself = nc.dram_tensor("pself", [NPC, 4], F32, kind="ExternalInput").ap()
    feat = nc.dram_tensor("feat", [NPC, FEAT], F32, kind="ExternalOutput").ap()
    dbg = None
    if debug:
        dbg = {
            name: nc.dram_tensor(
                f"dbg_{name}", [PT, SUP * w], F32, kind="ExternalOutput"
            ).ap()
            for name, w in DBG_SHAPES.items()
        }
    with tile.TileContext(nc) as tc, ExitStack() as ctx:
        io = ctx.enter_context(tc.tile_pool(name="io", bufs=2))
        kp = ctx.enter_context(tc.tile_pool(name="kspace", bufs=1))
        app = ctx.enter_context(tc.tile_pool(name="pairspace", bufs=1))
        const = ctx.enter_context(tc.tile_pool(name="const", bufs=1))
        half_pi = const.tile([PT, 1], F32, tag="half_pi")
        nc.gpsimd.memset(half_pi[:], HALF_PI)
        for s in range(SUP):
            build_supertile(
                nc, io, kp, app, s, pos4, idx16, pself, feat, half_pi, dbg=dbg
            )
    nc.compile()
    return nc


_NC_CACHE = None


def get_program():
    global _NC_CACHE
    if _NC_CACHE is None:
        _NC_CACHE = build_program()
    return _NC_CACHE


def make_in_maps(positions, species_idx, neighbor_idx):
    pos4 = np.zeros((NPAD, ROWE), np.float32)
    pos4[:N, :3] = positions
    pos4[:N, 3] = 2.0 * species_idx.astype(np.float32) - 1.0  # TYPESPIN[-1, 1]
    nbrK = np.zeros((NPAD, K), np.int32)
    nbrK[:N] = neighbor_idx.reshape(N, K)

    CQ = KG // GQ
    c_idx = np.arange(KG)
    g_of, k_of = c_idx // K, c_idx % K
    p = np.arange(PT)
    in_maps = []
    for c in range(NCORES):
        cb = c * NPC
        # idx value for (supertile s, chunk cidx, partition p):
        #   nbrK[cb + s*PT*G + p*G + g(cidx), k(cidx)]
        blocks = []
        for s in range(SUP):
            atoms = cb + s * PT * G + p[None, :] * G + g_of[:, None]  # [KG, PT]
            vals = nbrK[atoms, k_of[:, None]].astype(np.int16)  # [KG, PT]
            for q in range(GQ):
                flat = vals[q * CQ : (q + 1) * CQ, :].reshape(-1)  # i = cc*128+p
                wrapped = flat.reshape(-1, 16).T  # [16, NIDX/16]
                blocks.append(np.tile(wrapped, (PT // 16, 1)))
        idx16 = np.concatenate(blocks, axis=0)  # [SUP*GQ*PT, NIDX/16]
        in_maps.append(
            {
                "pos4": pos4,
                "idx16": np.ascontiguousarray(idx16),
                "pself": np.ascontiguousarray(pos4[cb : cb + NPC, :4]),
            }
        )
    return in_maps


def run(positions, species_idx, neighbor_idx, trace=False, trace_cores=None):
    nc = get_program()
    in_maps = make_in_maps(positions, species_idx, neighbor_idx)
    res = run_bass_kernel_spmd(
        nc,
        in_maps,
        core_ids=list(range(NCORES)),
        trace=trace,
        trace_cores=trace_cores,
    )
    out = np.concatenate([res.results[c]["feat"] for c in range(NCORES)], axis=0)
    return out[:N], res


def kernel(positions, species_idx, neighbor_idx):
    out, _ = run(positions, species_idx, neighbor_idx, trace=False)
    return out
